# revision 22
# baseline (speedup 1.0000x reference)
"""GRUCell fused kernel for Trainium2, data-parallel over 8 NeuronCores.

Strategy (v9, mixed precision):
  - Shard batch (16384) across 8 cores -> 2048 rows/core; replicate weights.
  - r/z/h gates run as fp8e4 DoubleRow matmuls: 2 K-rows per PE cell per
    cycle = 2x tensor throughput.  Acts scaled x16, weights x512 (TRN e4m3
    max 240); the 1/8192 unscale is folded into the PSUM-reading ACT op.
    i gate + epilogue in bf16.  Whole-scheme numerics: 1.244e-2 rel Fro
    error vs the 2e-2 budget (hardware reproduces the numpy model).
  - PE warmup matmuls on a zeroed tile release the HAM clock-gate to
    2.4 GHz before the first real data lands.
  - Merged input DMAs (one per logical tensor): the sync engine needs
    ~0.7us per DMA issue, so few large DMAs beat many small ones; jt0's
    weights are separate small DMAs ordered for earliest r-phase start.
  - Per output j-tile: phases r -> hg -> ig/z interleaved per batch-tile;
    PSUM ring A: r, hg, z / ring B: ig, so every PE phase transition waits
    only on an ACT op (sigmoid / identity-move), never on the DVE.
  - Epilogue per tile: r=sig(r_ps/8192+br) [ACT], hgm=(hg_ps/8192+bh)
    [ACT], m=hgm*r [DVE], s=(ig_ps+bi)+m [DVE stt], n=tanh(s) [ACT],
    d=h-n [GPSIMD for bt 0-2], z=sig(z_ps/8192+bz) [ACT], e=z*d, o=n+e
    [DVE, bf16 = 2x rate]; bf16 output, host upcasts.
"""

import os
import numpy as np
import ml_dtypes
from contextlib import ExitStack

import concourse.bass as bass
import concourse.tile as tile
from concourse import bacc, mybir
from concourse.bass_utils import run_bass_kernel_spmd

B, I, H = 16384, 512, 512
NCORES = 8
BL = B // NCORES          # 2048 rows per core
NB = 512                  # batch tile (matmul moving free dim)
NBT = BL // NB            # 4 batch tiles per core
P = 128                   # partitions
KT = I // P               # 4 k-tiles (128) per of x/h
KS = (I + H) // P         # 8 k-subtiles across the r/z contraction
JT = H // P               # 4 output j-tiles per gate

ASCALE = 16.0             # fp8 activation scale
WSCALE = 512.0            # fp8 weight scale
INV_SCALE = 1.0 / (ASCALE * WSCALE)

FP32 = mybir.dt.float32
BF16 = mybir.dt.bfloat16
FP8 = mybir.dt.float8e4

_cache = {}


def build_gru_bass():
    """Build (once) the SPMD Bass program for one core's shard."""
    if "nc" in _cache:
        return _cache["nc"]

    nc = bacc.Bacc(
        "TRN2",
        target_bir_lowering=False,
        debug=False,
        enable_asserts=False,
        num_devices=NCORES,
    )

    # feature-major activations, k-subtile-packed so each is ONE dma:
    # [p, kt, b] = act.T[kt*128 + p, b]
    xb = nc.dram_tensor("xb", [P, KT, BL], BF16, kind="ExternalInput").ap()
    hb = nc.dram_tensor("hb", [P, KT, BL], BF16, kind="ExternalInput").ap()
    x8 = nc.dram_tensor("x8", [P, 4, BL], FP8, kind="ExternalInput").ap()
    h8 = nc.dram_tensor("h8", [P, 4, BL], FP8, kind="ExternalInput").ap()
    # jt0 weights (small, issued first); jt1-3 merged per type.
    # fp8 DoubleRow weights: [(jt,) p, ks, m] = W.T[ks*128+p, jt*128+m]*512
    wr0 = nc.dram_tensor("wr0", [P, KS, P], FP8, kind="ExternalInput").ap()
    wz0 = nc.dram_tensor("wz0", [P, KS, P], FP8, kind="ExternalInput").ap()
    wh0 = nc.dram_tensor("wh0", [P, KT, P], FP8, kind="ExternalInput").ap()
    wi0 = nc.dram_tensor("wi0", [P, I], BF16, kind="ExternalInput").ap()
    wr123 = nc.dram_tensor("wr123", [P, 3, KS, P], FP8, kind="ExternalInput").ap()
    wz123 = nc.dram_tensor("wz123", [P, 3, KS, P], FP8, kind="ExternalInput").ap()
    wh123 = nc.dram_tensor("wh123", [P, 3, KT, P], FP8, kind="ExternalInput").ap()
    wi123 = nc.dram_tensor("wi123", [P, 3, I], BF16, kind="ExternalInput").ap()
    # bias columns: 0..3 b_r per j-tile, 4..7 b_z, 8..11 b_i, 12..15 b_h
    bias = nc.dram_tensor("bias", [P, 16], FP32, kind="ExternalInput").ap()
    outT = nc.dram_tensor("outT", [H, BL], BF16, kind="ExternalOutput").ap()

    ADD = mybir.AluOpType.add
    MULT = mybir.AluOpType.mult
    SUB = mybir.AluOpType.subtract
    SIG = mybir.ActivationFunctionType.Sigmoid
    TANH = mybir.ActivationFunctionType.Tanh
    IDENT = mybir.ActivationFunctionType.Identity
    DR = mybir.MatmulPerfMode.DoubleRow

    with tile.TileContext(nc) as tc, ExitStack() as ctx:
        wpool = ctx.enter_context(tc.tile_pool(name="weights", bufs=1))
        apool = ctx.enter_context(tc.tile_pool(name="acts", bufs=1))
        ppool = ctx.enter_context(tc.tile_pool(name="psum", bufs=1, space="PSUM"))
        epool = ctx.enter_context(tc.tile_pool(name="epi", bufs=2))

        # PE warmup: matmuls on a zeroed tile, no DMA dependency, so the
        # HAM clock-gate releases to 2.4 GHz before real data arrives.
        # Bank psB0 is first reused by ig-jt0, well after the warmup ends.
        warm = apool.tile([P, NB], BF16, tag="warm", name="warm")
        nc.gpsimd.memset(warm[:], 0.0)
        warm_ps = ppool.tile([P, NB], FP32, tag="psB0", name="warm_ps")
        for _ in range(10):
            nc.tensor.matmul(out=warm_ps[:], lhsT=warm[:, 0:P], rhs=warm[:],
                             start=True, stop=True)

        # ---- input DMAs, in first-use order, one per tensor ----
        bias_s = wpool.tile([P, 16], FP32, tag="bias", name="bias_s")
        wr0_s = wpool.tile([P, KS, P], FP8, tag="wr0", name="wr0_s")
        wz0_s = wpool.tile([P, KS, P], FP8, tag="wz0", name="wz0_s")
        wh0_s = wpool.tile([P, KT, P], FP8, tag="wh0", name="wh0_s")
        wi0_s = wpool.tile([P, I], BF16, tag="wi0", name="wi0_s")
        wr123_s = wpool.tile([P, 3, KS, P], FP8, tag="wr123", name="wr123_s")
        wz123_s = wpool.tile([P, 3, KS, P], FP8, tag="wz123", name="wz123_s")
        wh123_s = wpool.tile([P, 3, KT, P], FP8, tag="wh123", name="wh123_s")
        wi123_s = wpool.tile([P, 3, I], BF16, tag="wi123", name="wi123_s")
        x8_s = apool.tile([P, 4, BL], FP8, tag="x8", name="x8_s")
        h8_s = apool.tile([P, 4, BL], FP8, tag="h8", name="h8_s")
        xb_s = apool.tile([P, KT, BL], BF16, tag="xb", name="xb_s")
        hb_s = apool.tile([P, KT, BL], BF16, tag="hb", name="hb_s")

        nc.sync.dma_start(out=wr0_s[:], in_=wr0[:, :, :])
        nc.sync.dma_start(out=x8_s[:, 0:2, :], in_=x8[:, 0:2, :])
        nc.sync.dma_start(out=x8_s[:, 2:4, :], in_=x8[:, 2:4, :])
        nc.sync.dma_start(out=h8_s[:, 0:2, :], in_=h8[:, 0:2, :])
        nc.sync.dma_start(out=h8_s[:, 2:4, :], in_=h8[:, 2:4, :])
        nc.sync.dma_start(out=wh0_s[:], in_=wh0[:, :, :])
        nc.sync.dma_start(out=wz0_s[:], in_=wz0[:, :, :])
        nc.sync.dma_start(out=wi0_s[:], in_=wi0[:, :])
        nc.sync.dma_start(out=xb_s[:, 0:2, :], in_=xb[:, 0:2, :])
        nc.sync.dma_start(out=bias_s[:], in_=bias[:, :])
        nc.sync.dma_start(out=xb_s[:, 2:4, :], in_=xb[:, 2:4, :])
        nc.sync.dma_start(out=wr123_s[:], in_=wr123[:, :, :, :])
        nc.sync.dma_start(out=wh123_s[:], in_=wh123[:, :, :, :])
        nc.sync.dma_start(out=wz123_s[:], in_=wz123[:, :, :, :])
        nc.sync.dma_start(out=wi123_s[:], in_=wi123[:, :, :])
        nc.sync.dma_start(out=hb_s[:], in_=hb[:, :, :])

        # weight slice accessors (jt0 tiles vs merged jt1-3 tiles)
        def wr_sl(jt, ws):
            if jt == 0:
                return wr0_s[:, ws:ws + 2, :]
            return wr123_s[:, jt - 1, ws:ws + 2, :]

        def wz_sl(jt, ws):
            if jt == 0:
                return wz0_s[:, ws:ws + 2, :]
            return wz123_s[:, jt - 1, ws:ws + 2, :]

        def wh_sl(jt, ws):
            if jt == 0:
                return wh0_s[:, ws:ws + 2, :]
            return wh123_s[:, jt - 1, ws:ws + 2, :]

        def wi_sl(jt, kt):
            if jt == 0:
                return wi0_s[:, kt * P:(kt + 1) * P]
            return wi123_s[:, jt - 1, kt * P:(kt + 1) * P]

        # r/z DoubleRow chunks in DMA-arrival order: x first, then h.
        # chunk -> (acts tile, acts ks, weight ks)
        RZ_CHUNKS = [(x8_s, 0, 0), (x8_s, 2, 2), (h8_s, 0, 4), (h8_s, 2, 6)]
        HG_CHUNKS = [(h8_s, 0, 0), (h8_s, 2, 2)]

        def dr_phase(ps, w_sl, jt, chunks, bt_outer):
            nck = len(chunks)
            if bt_outer:
                for bt in range(NBT):
                    for kc in range(nck):
                        act, ks, ws = chunks[kc]
                        nc.tensor.matmul(
                            out=ps[bt][:], lhsT=w_sl(jt, ws),
                            rhs=act[:, ks:ks + 2, bass.ts(bt, NB)],
                            start=(kc == 0), stop=(kc == nck - 1),
                            perf_mode=DR)
            else:
                for kc in range(nck):
                    act, ks, ws = chunks[kc]
                    for bt in range(NBT):
                        nc.tensor.matmul(
                            out=ps[bt][:], lhsT=w_sl(jt, ws),
                            rhs=act[:, ks:ks + 2, bass.ts(bt, NB)],
                            start=(kc == 0), stop=(kc == nck - 1),
                            perf_mode=DR)

        # ---- main loop over output j-tiles ----
        for jt in range(JT):
            j0 = jt * P
            first = jt == 0

            # phase r: fp8 DR, K = I+H (banks A)
            r_ps = [ppool.tile([P, NB], FP32, tag=f"psA{bt}",
                               name=f"r_ps_{jt}_{bt}") for bt in range(NBT)]
            dr_phase(r_ps, wr_sl, jt, RZ_CHUNKS, bt_outer=not first)
            r_s = [None] * NBT
            for bt in range(NBT):
                r_s[bt] = epool.tile([P, NB], BF16, tag=f"r_s{bt}",
                                     name=f"r_s_{jt}_{bt}")
                nc.scalar.activation(out=r_s[bt][:], in_=r_ps[bt][:], func=SIG,
                                     bias=bias_s[:, jt:jt + 1], scale=INV_SCALE)

            # phase hg: fp8 DR, K = H (banks A, freed per-bank by the r
            # sigmoids)
            hg_ps = [ppool.tile([P, NB], FP32, tag=f"psA{bt}",
                                name=f"hg_ps_{jt}_{bt}") for bt in range(NBT)]
            dr_phase(hg_ps, wh_sl, jt, HG_CHUNKS, bt_outer=True)
            # hgm = hg/8192 + b_h (ACT move, frees banks A); m = hgm * r
            m = [None] * NBT
            for bt in range(NBT):
                hgm = epool.tile([P, NB], BF16, tag=f"hgm{bt}",
                                 name=f"hgm_{jt}_{bt}")
                nc.scalar.activation(out=hgm[:], in_=hg_ps[bt][:], func=IDENT,
                                     bias=bias_s[:, 12 + jt:13 + jt],
                                     scale=INV_SCALE)
                m[bt] = epool.tile([P, NB], BF16, tag=f"m{bt}",
                                   name=f"m_{jt}_{bt}")
                nc.vector.tensor_tensor(out=m[bt][:], in0=hgm[:],
                                        in1=r_s[bt][:], op=MULT)

            # phases ig (banks B) and z (banks A, freed per-bank by hgm):
            # interleaved per batch-tile for jt>=1 so each tile's epilogue
            # drains during the remaining matmuls and the post-stream tail
            # is one tile's z_s -> e -> o chain.
            ig_ps = [ppool.tile([P, NB], FP32, tag=f"psB{bt}",
                                name=f"ig_ps_{jt}_{bt}") for bt in range(NBT)]
            z_ps = [ppool.tile([P, NB], FP32, tag=f"psA{bt}",
                               name=f"z_ps_{jt}_{bt}") for bt in range(NBT)]

            def ig_mms(bt):
                for kt in range(KT):
                    nc.tensor.matmul(
                        out=ig_ps[bt][:], lhsT=wi_sl(jt, kt),
                        rhs=xb_s[:, kt, bass.ts(bt, NB)],
                        start=(kt == 0), stop=(kt == KT - 1))

            def z_mms(bt):
                for kc in range(len(RZ_CHUNKS)):
                    act, ks, ws = RZ_CHUNKS[kc]
                    nc.tensor.matmul(
                        out=z_ps[bt][:], lhsT=wz_sl(jt, ws),
                        rhs=act[:, ks:ks + 2, bass.ts(bt, NB)],
                        start=(kc == 0), stop=(kc == len(RZ_CHUNKS) - 1),
                        perf_mode=DR)

            def ig_epi(bt):
                # s = (i_gate + b_i) + m; n = tanh(s); d = h - n
                bsl = bass.ts(bt, NB)
                s = epool.tile([P, NB], BF16, tag=f"s{bt}", name=f"s_{jt}_{bt}")
                nc.vector.scalar_tensor_tensor(
                    out=s[:], in0=ig_ps[bt][:],
                    scalar=bias_s[:, 8 + jt:9 + jt],
                    in1=m[bt][:], op0=ADD, op1=ADD)
                n[bt] = epool.tile([P, NB], BF16, tag=f"n{bt}",
                                   name=f"n_{jt}_{bt}")
                nc.scalar.activation(out=n[bt][:], in_=s[:], func=TANH)
                d[bt] = epool.tile([P, NB], BF16, tag=f"d{bt}",
                                   name=f"d_{jt}_{bt}")
                nc.vector.tensor_tensor(
                    out=d[bt][:], in0=hb_s[:, jt, bsl], in1=n[bt][:], op=SUB)

            def z_epi(bt):
                # z = sig(z_ps/8192 + b_z); h' = n + z*d
                bsl = bass.ts(bt, NB)
                z_s = epool.tile([P, NB], BF16, tag=f"z_s{bt}",
                                 name=f"z_s_{jt}_{bt}")
                nc.scalar.activation(out=z_s[:], in_=z_ps[bt][:], func=SIG,
                                     bias=bias_s[:, 4 + jt:5 + jt],
                                     scale=INV_SCALE)
                e = epool.tile([P, NB], BF16, tag=f"e{bt}", name=f"e_{jt}_{bt}")
                nc.vector.tensor_tensor(
                    out=e[:], in0=z_s[:], in1=d[bt][:], op=MULT)
                o = epool.tile([P, NB], BF16, tag=f"o{bt}", name=f"o_{jt}_{bt}")
                nc.vector.tensor_tensor(
                    out=o[:], in0=n[bt][:], in1=e[:], op=ADD)
                nc.sync.dma_start(out=outT[j0:j0 + P, bsl], in_=o[:])

            n = [None] * NBT
            d = [None] * NBT
            if first:
                # DMA-arrival-tolerant jt0: the z phase runs BEFORE ig — it
                # needs only x8/h8 (+tiny wz0), already resident, while the
                # 2MB xb DMA is still in flight.  ig is k-tile-outer.
                dr_phase(z_ps, wz_sl, jt, RZ_CHUNKS, bt_outer=True)
                z_s0 = [None] * NBT
                for bt in range(NBT):
                    z_s0[bt] = epool.tile([P, NB], BF16, tag=f"z_s{bt}",
                                          name=f"z_s_{jt}_{bt}")
                    nc.scalar.activation(out=z_s0[bt][:], in_=z_ps[bt][:],
                                         func=SIG,
                                         bias=bias_s[:, 4 + jt:5 + jt],
                                         scale=INV_SCALE)
                for kt in range(KT):
                    for bt in range(NBT):
                        nc.tensor.matmul(
                            out=ig_ps[bt][:], lhsT=wi_sl(jt, kt),
                            rhs=xb_s[:, kt, bass.ts(bt, NB)],
                            start=(kt == 0), stop=(kt == KT - 1))
                for bt in range(NBT):
                    ig_epi(bt)
                for bt in range(NBT):
                    bsl = bass.ts(bt, NB)
                    e = epool.tile([P, NB], BF16, tag=f"e{bt}",
                                   name=f"e_{jt}_{bt}")
                    nc.vector.tensor_tensor(
                        out=e[:], in0=z_s0[bt][:], in1=d[bt][:], op=MULT)
                    o = epool.tile([P, NB], BF16, tag=f"o{bt}",
                                   name=f"o_{jt}_{bt}")
                    nc.vector.tensor_tensor(
                        out=o[:], in0=n[bt][:], in1=e[:], op=ADD)
                    nc.sync.dma_start(out=outT[j0:j0 + P, bsl], in_=o[:])
            else:
                # bt3's long chain (s->n->d) is hoisted to the front of the
                # interleave and its short z chain (z_s->e->o) to the very
                # end, so the post-stream tail is ~2us instead of ~6.
                last = NBT - 1
                ig_mms(last)
                ig_epi(last)
                for bt in range(NBT - 1):
                    ig_mms(bt)
                    ig_epi(bt)
                    z_mms(bt)
                    z_epi(bt)
                z_mms(last)
                z_epi(last)

    nc.compile()
    _cache["nc"] = nc
    return nc


def _pack_weights(W_gate, b_gate, W_i, b_i, W_h, b_h):
    bf16 = ml_dtypes.bfloat16
    fp8 = ml_dtypes.float8_e4m3

    def pack_bf16(WT):  # [I, H] -> [JT, P, I] with [jt, p, kt*128+m]
        a = WT.reshape(KT, P, JT, P).transpose(2, 1, 0, 3).reshape(JT, P, I)
        return np.ascontiguousarray(a.astype(bf16))

    def pack_fp8(WT):   # [K, 512] -> [JT, P, K/128, P]
        ks = WT.shape[0] // P
        a = np.clip(WT * WSCALE, -240.0, 240.0)
        a = a.reshape(ks, P, JT, P).transpose(2, 1, 0, 3)
        return np.ascontiguousarray(a.astype(fp8))

    wi = pack_bf16(W_i.T)
    wr = pack_fp8(W_gate[:H].T)
    wz = pack_fp8(W_gate[H:].T)
    wh = pack_fp8(W_h.T)
    biasp = np.concatenate([
        b_gate[:H].reshape(JT, P).T,
        b_gate[H:].reshape(JT, P).T,
        b_i.reshape(JT, P).T,
        b_h.reshape(JT, P).T,
    ], axis=1).astype(np.float32)

    def split(w):  # [JT, ...] -> jt0 [P, ...] and jt1-3 [P, 3, ...]
        w0 = np.ascontiguousarray(w[0])
        w123 = np.ascontiguousarray(np.moveaxis(w[1:], 0, 1))
        return w0, w123

    wr0, wr123 = split(wr)
    wz0, wz123 = split(wz)
    wh0, wh123 = split(wh)
    wi0, wi123 = split(wi)
    return (wr0, wr123, wz0, wz123, wh0, wh123, wi0, wi123,
            np.ascontiguousarray(biasp))


def kernel(input, hidden, W_gate, b_gate, W_i, b_i, W_h, b_h):
    input = np.asarray(input, dtype=np.float32)
    hidden = np.asarray(hidden, dtype=np.float32)
    W_gate = np.asarray(W_gate, dtype=np.float32)
    b_gate = np.asarray(b_gate, dtype=np.float32)
    W_i = np.asarray(W_i, dtype=np.float32)
    b_i = np.asarray(b_i, dtype=np.float32)
    W_h = np.asarray(W_h, dtype=np.float32)
    b_h = np.asarray(b_h, dtype=np.float32)

    nc = build_gru_bass()
    (wr0, wr123, wz0, wz123, wh0, wh123, wi0, wi123, biasp) = _pack_weights(
        W_gate, b_gate, W_i, b_i, W_h, b_h)

    bf16 = ml_dtypes.bfloat16
    fp8 = ml_dtypes.float8_e4m3

    def pack8(aT):  # [512, BL] fp32 -> [P, 4, BL] fp8 (scaled)
        a = np.clip(aT * ASCALE, -240.0, 240.0)
        a = a.reshape(4, P, BL).transpose(1, 0, 2)
        return np.ascontiguousarray(a.astype(fp8))

    def packb(aT):  # [512, BL] fp32 -> [P, KT, BL] bf16
        a = aT.reshape(KT, P, BL).transpose(1, 0, 2)
        return np.ascontiguousarray(a.astype(bf16))

    in_maps = []
    for c in range(NCORES):
        sl = slice(c * BL, (c + 1) * BL)
        xT = np.ascontiguousarray(input[sl].T)
        hT = np.ascontiguousarray(hidden[sl].T)
        in_maps.append({
            "xb": packb(xT),
            "hb": packb(hT),
            "x8": pack8(xT),
            "h8": pack8(hT),
            "wr0": wr0, "wr123": wr123,
            "wz0": wz0, "wz123": wz123,
            "wh0": wh0, "wh123": wh123,
            "wi0": wi0, "wi123": wi123,
            "bias": biasp,
        })

    res = run_bass_kernel_spmd(
        nc, in_maps, list(range(NCORES)),
        trace=bool(int(os.environ.get("GRU_TRACE", "0"))),
    )
    out = np.empty((B, H), dtype=np.float32)
    for c in range(NCORES):
        out[c * BL:(c + 1) * BL, :] = res.results[c]["outT"].astype(np.float32).T
    if res.exec_time_ns is not None:
        kernel.last_exec_time_ns = res.exec_time_ns
        kernel.last_results = res
    return out


kernel.last_exec_time_ns = None
kernel.last_results = None
